# revision 6
# baseline (speedup 1.0000x reference)
"""Trainium2 Bass kernel for nn_AttentionBlock (B=2, T=2048, D=1024, H=16).

Sharding: 8 cores = 2 batches x 4 query-blocks of 512 tokens.
Each core: recomputes K/V projection for its batch (full 2048 kv tokens),
Q projection + attention + FFN for its 512-token query block. Zero
collectives; host gathers the 8 [512, 1024] output shards.

All matmuls run as float32r (full PE rate for N>=256, fp32 storage).
Orientation scheme keeps everything transposed so no on-device
transposes are needed:
  kT/qT: [head_dim, tokens]    logitsT: [kv, q]   p = exp(logitsT)*maskT
  v:     [kv, head_dim(+ones)] av: [head_dim+1, q] (row 64 = softmax denom)
  hT:    [d, q]                aT(ffn): [dff, q]   out: [q, d]
"""

import sys

for _p in ("/opt/trn_rl_repo",):
    if _p not in sys.path:
        sys.path.insert(0, _p)

import numpy as np

B, T, D, H = 2, 2048, 1024, 16
DH = D // H  # 64
DFF = 4 * D  # 4096
P = 128
TQ = 512  # query tokens per core
NCORES = 8
DO = D // P  # 8 d-chunks
SC = T // P  # 16 kv chunks
MC = DFF // P  # 32 ff chunks


def _build():
    import concourse.bass as bass
    import concourse.mybir as mybir
    import concourse.tile as tile
    from concourse import bacc

    f32 = mybir.dt.float32
    f32r = mybir.dt.float32r
    AF = mybir.ActivationFunctionType

    nc = bacc.Bacc("TRN2", target_bir_lowering=False)

    # ---- I/O ----  (f32r tensors hold plain fp32 bits; np dtype is float32)
    xT = nc.dram_tensor("xT", [D, T], f32r, kind="ExternalInput")
    xqT = nc.dram_tensor("xqT", [D, TQ], f32r, kind="ExternalInput")
    x_res = nc.dram_tensor("x_res", [TQ, D], f32, kind="ExternalInput")
    maskT = nc.dram_tensor("maskT", [T, TQ], f32, kind="ExternalInput")
    wqT = nc.dram_tensor("wqT", [D, D], f32r, kind="ExternalInput")
    wkT = nc.dram_tensor("wkT", [D, D], f32r, kind="ExternalInput")
    wvT = nc.dram_tensor("wvT", [D, D], f32r, kind="ExternalInput")
    w1T = nc.dram_tensor("w1T", [D, DFF], f32r, kind="ExternalInput")
    w2T = nc.dram_tensor("w2T", [DFF, D], f32r, kind="ExternalInput")
    b1c = nc.dram_tensor("b1c", [P, MC], f32, kind="ExternalInput")
    ones_in = nc.dram_tensor("ones_in", [P, DH], f32r, kind="ExternalInput")
    out = nc.dram_tensor("out", [TQ, D], f32, kind="ExternalOutput")

    def r(ap):  # fp32 view -> float32r view (same bits)
        return ap.bitcast(f32r)

    with tile.TileContext(nc) as tc, nc.allow_low_precision(
        reason="float32r is full fp32 storage; PSUM accumulation stays fp32"
    ):
        with (
            tc.tile_pool(name="dram", bufs=1, space="DRAM") as dram,
            tc.tile_pool(name="consts", bufs=1) as consts,
            tc.tile_pool(name="resident", bufs=1) as resident,
            tc.tile_pool(name="psum", bufs=4, space="PSUM") as psum,
            tc.tile_pool(name="psum_av", bufs=2, space="PSUM") as psum_av,
            tc.tile_pool(name="psum_b", bufs=2, space="PSUM") as psum_b,
        ):
            kT_d = dram.tile([D, T], f32r)
            qT_d = dram.tile([D, TQ], f32r)
            v_d = dram.tile([T, D], f32r)

            ones_full = consts.tile([P, DH], f32r)
            nc.sync.dma_start(ones_full, ones_in[:, :])
            ones_sb = ones_full[0:1, :]
            b1_sb = consts.tile([P, MC], f32)
            nc.sync.dma_start(b1_sb, b1c[:, :])

            maskT_sb = resident.tile([P, SC, TQ], f32)
            nc.sync.dma_start(maskT_sb, maskT[:, :].rearrange("(o p) t -> p o t", p=P))
            # hT: written per-head (f32r views), then residual-add; read by ff1
            hT_sb = resident.tile([P, DO, TQ], f32r)

            # ================= Phase 1: Q/K/V projections =================
            with tc.tile_pool(name="ph1", bufs=2) as ph1, tc.tile_pool(
                name="ph1w", bufs=1
            ) as ph1w:
                wk_sb = ph1w.tile([P, DO, D], f32r)
                nc.sync.dma_start(wk_sb, wkT[:, :].rearrange("(o p) e -> p o e", p=P))
                wv_sb = ph1w.tile([P, DO, D], f32r)
                nc.sync.dma_start(wv_sb, wvT[:, :].rearrange("(o p) e -> p o e", p=P))
                wq_sb = ph1w.tile([P, DO, D], f32r)
                nc.sync.dma_start(wq_sb, wqT[:, :].rearrange("(o p) e -> p o e", p=P))

                xq_sb = ph1w.tile([P, DO, TQ], f32r)
                nc.sync.dma_start(xq_sb, xqT[:, :].rearrange("(o p) t -> p o t", p=P))

                # Q projection: qT[he, tq] ; he chunks of 128
                for hc in range(DO):
                    ps = psum.tile([P, TQ], f32, tag="ps_main")
                    for dc in range(DO):
                        nc.tensor.matmul(
                            ps,
                            wq_sb[:, dc, hc * P : (hc + 1) * P],
                            xq_sb[:, dc, :],
                            start=(dc == 0),
                            stop=(dc == DO - 1),
                        )
                    qv = ph1.tile([P, TQ], f32r, tag="qstage")
                    nc.any.tensor_copy(qv, ps)
                    nc.sync.dma_start(qT_d[hc * P : (hc + 1) * P, :], qv)

                # K/V projections, streaming x by 512-token column blocks
                for s4 in range(T // TQ):
                    xb = ph1.tile([P, DO, TQ], f32r, tag="xblk")
                    nc.sync.dma_start(
                        xb,
                        xT[:, :].rearrange("(o p) t -> p o t", p=P)[
                            :, :, s4 * TQ : (s4 + 1) * TQ
                        ],
                    )
                    # K: kT[he, s]
                    for hc in range(DO):
                        ps = psum.tile([P, TQ], f32, tag="ps_main")
                        for dc in range(DO):
                            nc.tensor.matmul(
                                ps,
                                wk_sb[:, dc, hc * P : (hc + 1) * P],
                                xb[:, dc, :],
                                start=(dc == 0),
                                stop=(dc == DO - 1),
                            )
                        kv = ph1.tile([P, TQ], f32r, tag="kstage")
                        nc.any.tensor_copy(kv, ps)
                        nc.sync.dma_start(
                            kT_d[hc * P : (hc + 1) * P, s4 * TQ : (s4 + 1) * TQ], kv
                        )
                    # V: v[s, he]; s chunks of 128 within this 512 block
                    for sl in range(TQ // P):
                        sc = s4 * (TQ // P) + sl
                        for n2 in range(D // TQ):
                            ps = psum.tile([P, TQ], f32, tag="ps_main")
                            for dc in range(DO):
                                nc.tensor.matmul(
                                    ps,
                                    xb[:, dc, sl * P : (sl + 1) * P],
                                    wv_sb[:, dc, n2 * TQ : (n2 + 1) * TQ],
                                    start=(dc == 0),
                                    stop=(dc == DO - 1),
                                )
                            vv = ph1.tile([P, TQ], f32r, tag="vstage")
                            nc.any.tensor_copy(vv, ps)
                            nc.sync.dma_start(
                                v_d[sc * P : (sc + 1) * P, n2 * TQ : (n2 + 1) * TQ], vv
                            )

            # ================= Phase 2: attention per head =================
            with tc.tile_pool(name="ph2", bufs=2) as ph2:
                for h in range(H):
                    kT_h = ph2.tile([DH, T], f32r, tag="kT_h")
                    nc.sync.dma_start(kT_h, kT_d[h * DH : (h + 1) * DH, :])
                    q_h = ph2.tile([DH, TQ], f32r, tag="q_h")
                    nc.sync.dma_start(q_h, qT_d[h * DH : (h + 1) * DH, :])
                    v_ones = ph2.tile([P, SC, DH + 1], f32r, tag="v_ones")
                    nc.sync.dma_start(v_ones[:, :, DH], ones_full[:, 0:SC])
                    nc.sync.dma_start(
                        v_ones[:, :, 0:DH],
                        v_d.rearrange("(o p) e -> p o e", p=P)[
                            :, :, h * DH : (h + 1) * DH
                        ],
                    )

                    p_sb = ph2.tile([P, SC, TQ], f32r, tag="p_sb")
                    for sc in range(SC):
                        ps_l = psum.tile([P, TQ], f32, tag="ps_main")
                        nc.tensor.matmul(
                            ps_l,
                            kT_h[:, sc * P : (sc + 1) * P],
                            q_h,
                            start=True,
                            stop=True,
                        )
                        # p = exp(logits / sqrt(dh)) * mask
                        ptmp = ph2.tile([P, TQ], f32, tag="ptmp")
                        nc.scalar.activation(
                            ptmp, ps_l, AF.Exp, scale=float(DH**-0.5)
                        )
                        nc.vector.tensor_mul(
                            p_sb[:, sc, :], ptmp, maskT_sb[:, sc, :]
                        )

                    ps_av = psum_av.tile([DH + 1, TQ], f32, tag="ps_av")
                    for sc in range(SC):
                        nc.tensor.matmul(
                            ps_av,
                            v_ones[:, sc, :],
                            p_sb[:, sc, :],
                            start=(sc == 0),
                            stop=(sc == SC - 1),
                        )
                    av_sb = ph2.tile([DH + 1, TQ], f32, tag="av_sb")
                    nc.any.tensor_copy(av_sb, ps_av)
                    recip = ph2.tile([1, TQ], f32r, tag="recip")
                    nc.vector.reciprocal(recip, av_sb[DH : DH + 1, :])
                    ps_bc = psum_b.tile([DH, TQ], f32, tag="ps_bc")
                    nc.tensor.matmul(ps_bc, ones_sb, recip, start=True, stop=True)
                    # normalized attn head output -> hT[(h%2)*64:+64, h//2, :]
                    nc.vector.tensor_mul(
                        hT_sb[(h % 2) * DH : (h % 2) * DH + DH, h // 2, :],
                        av_sb[0:DH, :],
                        ps_bc,
                    )

            # ================= Phase 3: residual + FFN =================
            with tc.tile_pool(name="ph3", bufs=3) as ph3, tc.tile_pool(
                name="ph3a", bufs=1
            ) as ph3a:
                xq_sb2 = ph3a.tile([P, DO, TQ], f32r)
                nc.sync.dma_start(xq_sb2, xqT[:, :].rearrange("(o p) t -> p o t", p=P))
                for dc in range(DO):
                    nc.vector.tensor_add(
                        hT_sb[:, dc, :], hT_sb[:, dc, :], xq_sb2[:, dc, :]
                    )

                # ff1: aT[m, tq] = gelu(w1T.T @ hT + b1)
                aT_sb = ph3a.tile([P, MC, TQ], f32r)
                w1r = w1T[:, :].rearrange("(o p) m -> p o m", p=P)
                for mc in range(MC):
                    w1_sb = ph3.tile([P, DO, P], f32r, tag="w1_sb")
                    nc.sync.dma_start(w1_sb, w1r[:, :, mc * P : (mc + 1) * P])
                    ps = psum.tile([P, TQ], f32, tag="ps_main")
                    for dc in range(DO):
                        nc.tensor.matmul(
                            ps,
                            w1_sb[:, dc, :],
                            hT_sb[:, dc, :],
                            start=(dc == 0),
                            stop=(dc == DO - 1),
                        )
                    nc.scalar.activation(
                        aT_sb[:, mc, :], ps, AF.Gelu, bias=b1_sb[:, mc : mc + 1]
                    )

                # ff2: out[tq, d] = aT.T @ w2T + x_res
                xres_r = x_res[:, :].rearrange("(o p) d -> p o d", p=P)
                w2r = w2T[:, :].rearrange("(o p) d -> p o d", p=P)
                for tqc in range(TQ // P):
                    xr_sb = ph3.tile([P, D], f32, tag="xr_sb")
                    nc.sync.dma_start(xr_sb, xres_r[:, tqc, :])
                    for n2 in range(D // TQ):
                        ps = psum.tile([P, TQ], f32, tag="ps_main")
                        for mc in range(MC):
                            w2_sb = ph3.tile([P, TQ], f32r, tag="w2_sb")
                            nc.sync.dma_start(
                                w2_sb, w2r[:, mc, n2 * TQ : (n2 + 1) * TQ]
                            )
                            nc.tensor.matmul(
                                ps,
                                aT_sb[:, mc, tqc * P : (tqc + 1) * P],
                                w2_sb,
                                start=(mc == 0),
                                stop=(mc == MC - 1),
                            )
                        o_sb = ph3.tile([P, TQ], f32, tag="o_sb")
                        nc.vector.tensor_add(
                            o_sb, ps, xr_sb[:, n2 * TQ : (n2 + 1) * TQ]
                        )
                        nc.sync.dma_start(
                            out[tqc * P : (tqc + 1) * P, n2 * TQ : (n2 + 1) * TQ],
                            o_sb,
                        )

    nc.compile()
    return nc


_nc_cache = None


def _get_nc():
    global _nc_cache
    if _nc_cache is None:
        _nc_cache = _build()
    return _nc_cache


def _prepare_in_maps(x, mask, wq, wk, wv, w1, b1, w2, b2):
    f = np.float32
    c = np.ascontiguousarray
    wqT = c(wq.transpose(2, 0, 1).reshape(D, D).astype(f))
    wkT = c(wk.transpose(2, 0, 1).reshape(D, D).astype(f))
    wvT = c(wv.transpose(2, 0, 1).reshape(D, D).astype(f))
    w1T = c(w1.T.astype(f))
    w2T = c(w2.T.astype(f))
    b1c = c(b1.reshape(MC, P).T.astype(f))
    shared = dict(wqT=wqT, wkT=wkT, wvT=wvT, w1T=w1T, w2T=w2T, b1c=b1c,
                  ones_in=np.ones((P, DH), np.float32))

    in_maps = []
    for core in range(NCORES):
        b, jq = divmod(core, 4)
        q0 = jq * TQ
        xT_b = c(x[b].T.astype(f))
        m = dict(shared)
        m["xT"] = xT_b
        m["xqT"] = c(xT_b[:, q0 : q0 + TQ])
        m["x_res"] = c((x[b, q0 : q0 + TQ, :] + b2[None, :]).astype(f))
        m["maskT"] = c(mask[q0 : q0 + TQ, :].T.astype(f))
        in_maps.append(m)
    return in_maps


def _run(inputs, trace=False):
    from concourse import bass_utils

    nc = _get_nc()
    in_maps = _prepare_in_maps(**inputs)
    res = bass_utils.run_bass_kernel_spmd(
        nc, in_maps, core_ids=list(range(NCORES)), trace=trace
    )
    out = np.empty((B, T, D), np.float32)
    for core in range(NCORES):
        b, jq = divmod(core, 4)
        out[b, jq * TQ : (jq + 1) * TQ, :] = res.results[core]["out"]
    return out, res


def kernel(**inputs):
    inputs = {k: np.asarray(v) for k, v in inputs.items()}
    out, _ = _run(inputs, trace=False)
    return out


# revision 8
# speedup vs baseline: 1.2508x; 1.2508x over previous
"""Trainium2 Bass kernel for nn_AttentionBlock (B=2, T=2048, D=1024, H=16).

Sharding: 8 cores = 2 batches x 4 query-blocks of 512 tokens.
Each core: recomputes K/V projection for its batch (full 2048 kv tokens),
Q projection + attention + FFN for its 512-token query block. Zero
collectives; host gathers the 8 [512, 1024] output shards.

Matmul inputs are bf16 (1 cycle/row on PE, fp32 PSUM accumulation);
residual paths stay fp32. Orientation scheme keeps everything transposed
so no on-device transposes are needed:
  kT/qT: [head_dim, tokens]    logitsT: [kv, q]   p = exp(logitsT)*maskT
  v:     [kv, head_dim(+ones)] av: [head_dim+1, q] (row 64 = softmax denom)
  hT:    [d, q]                aT(ffn): [dff, q]   out: [q, d]
Softmax denominators for all 16 heads are normalized in one batch at the
end of attention (single RECIPROCAL + 8 selector-matmul broadcasts).
"""

import sys

for _p in ("/opt/trn_rl_repo",):
    if _p not in sys.path:
        sys.path.insert(0, _p)

import ml_dtypes
import numpy as np

B, T, D, H = 2, 2048, 1024, 16
DH = D // H  # 64
DFF = 4 * D  # 4096
P = 128
TQ = 512  # query tokens per core
NCORES = 8
DO = D // P  # 8 d-chunks
SC = T // P  # 16 kv chunks
MC = DFF // P  # 32 ff chunks

BF16 = ml_dtypes.bfloat16


def _build():
    import concourse.bass as bass
    import concourse.mybir as mybir
    import concourse.tile as tile
    from concourse import bacc

    f32 = mybir.dt.float32
    f32r = mybir.dt.float32r
    bf = mybir.dt.bfloat16
    AF = mybir.ActivationFunctionType

    nc = bacc.Bacc("TRN2", target_bir_lowering=False)

    # ---- I/O ----
    xT = nc.dram_tensor("xT", [D, T], bf, kind="ExternalInput")
    xqT = nc.dram_tensor("xqT", [D, TQ], bf, kind="ExternalInput")
    x_res = nc.dram_tensor("x_res", [TQ, D], f32, kind="ExternalInput")
    maskT = nc.dram_tensor("maskT", [T, TQ], bf, kind="ExternalInput")
    wqT = nc.dram_tensor("wqT", [D, D], bf, kind="ExternalInput")
    wkT = nc.dram_tensor("wkT", [D, D], bf, kind="ExternalInput")
    wvT = nc.dram_tensor("wvT", [D, D], bf, kind="ExternalInput")
    w1T = nc.dram_tensor("w1T", [D, DFF], bf, kind="ExternalInput")
    w2T = nc.dram_tensor("w2T", [DFF, D], bf, kind="ExternalInput")
    b1c = nc.dram_tensor("b1c", [P, MC], f32, kind="ExternalInput")
    # sel[h, dc*128+p] = 1 if head-of-partition-p-in-chunk-dc == h (f32r path)
    sel_in = nc.dram_tensor("sel_in", [H, D], f32r, kind="ExternalInput")
    out = nc.dram_tensor("out", [TQ, D], f32, kind="ExternalOutput")

    with tile.TileContext(nc) as tc, nc.allow_low_precision(
        reason="bf16 matmul inputs; all accumulation in fp32 PSUM"
    ):
        with (
            tc.tile_pool(name="dram", bufs=1, space="DRAM") as dram,
            tc.tile_pool(name="consts", bufs=1) as consts,
            tc.tile_pool(name="resident", bufs=1) as resident,
            tc.tile_pool(name="psum", bufs=4, space="PSUM") as psum,
            tc.tile_pool(name="psum_av", bufs=2, space="PSUM") as psum_av,
            tc.tile_pool(name="psum_b", bufs=2, space="PSUM") as psum_b,
        ):
            kT_d = dram.tile([D, T], bf)
            qT_d = dram.tile([D, TQ], bf)
            v_d = dram.tile([T, D], bf)

            sel_sb = consts.tile([H, D], f32r)
            nc.sync.dma_start(sel_sb, sel_in[:, :])
            b1_sb = consts.tile([P, MC], f32)
            nc.sync.dma_start(b1_sb, b1c[:, :])

            maskT_sb = resident.tile([P, SC, TQ], bf)
            nc.sync.dma_start(maskT_sb, maskT[:, :].rearrange("(o p) t -> p o t", p=P))
            # hT: unnormalized per-head attn outputs, then normalized + residual
            hT_sb = resident.tile([P, DO, TQ], bf)
            denom_sb = resident.tile([H, TQ], f32)

            # ================= Phase 1: Q/K/V projections =================
            with tc.tile_pool(name="ph1", bufs=2) as ph1, tc.tile_pool(
                name="ph1w", bufs=1
            ) as ph1w:
                wk_sb = ph1w.tile([P, DO, D], bf)
                nc.sync.dma_start(wk_sb, wkT[:, :].rearrange("(o p) e -> p o e", p=P))
                wv_sb = ph1w.tile([P, DO, D], bf)
                nc.sync.dma_start(wv_sb, wvT[:, :].rearrange("(o p) e -> p o e", p=P))
                wq_sb = ph1w.tile([P, DO, D], bf)
                nc.sync.dma_start(wq_sb, wqT[:, :].rearrange("(o p) e -> p o e", p=P))

                xq_sb = ph1w.tile([P, DO, TQ], bf)
                nc.sync.dma_start(xq_sb, xqT[:, :].rearrange("(o p) t -> p o t", p=P))

                # Q projection: qT[he, tq] ; he chunks of 128
                for hc in range(DO):
                    ps = psum.tile([P, TQ], f32, tag="ps_main")
                    for dc in range(DO):
                        nc.tensor.matmul(
                            ps,
                            wq_sb[:, dc, hc * P : (hc + 1) * P],
                            xq_sb[:, dc, :],
                            start=(dc == 0),
                            stop=(dc == DO - 1),
                        )
                    qv = ph1.tile([P, TQ], bf, tag="qstage")
                    nc.any.tensor_copy(qv, ps)
                    nc.sync.dma_start(qT_d[hc * P : (hc + 1) * P, :], qv)

                # K/V projections, streaming x by 512-token column blocks
                for s4 in range(T // TQ):
                    xb = ph1.tile([P, DO, TQ], bf, tag="xblk")
                    nc.sync.dma_start(
                        xb,
                        xT[:, :].rearrange("(o p) t -> p o t", p=P)[
                            :, :, s4 * TQ : (s4 + 1) * TQ
                        ],
                    )
                    # K: kT[he, s]
                    for hc in range(DO):
                        ps = psum.tile([P, TQ], f32, tag="ps_main")
                        for dc in range(DO):
                            nc.tensor.matmul(
                                ps,
                                wk_sb[:, dc, hc * P : (hc + 1) * P],
                                xb[:, dc, :],
                                start=(dc == 0),
                                stop=(dc == DO - 1),
                            )
                        kv = ph1.tile([P, TQ], bf, tag="kstage")
                        nc.any.tensor_copy(kv, ps)
                        nc.sync.dma_start(
                            kT_d[hc * P : (hc + 1) * P, s4 * TQ : (s4 + 1) * TQ], kv
                        )
                    # V: v[s, he]; s chunks of 128 within this 512 block
                    for sl in range(TQ // P):
                        sc = s4 * (TQ // P) + sl
                        for n2 in range(D // TQ):
                            ps = psum.tile([P, TQ], f32, tag="ps_main")
                            for dc in range(DO):
                                nc.tensor.matmul(
                                    ps,
                                    xb[:, dc, sl * P : (sl + 1) * P],
                                    wv_sb[:, dc, n2 * TQ : (n2 + 1) * TQ],
                                    start=(dc == 0),
                                    stop=(dc == DO - 1),
                                )
                            vv = ph1.tile([P, TQ], bf, tag="vstage")
                            nc.any.tensor_copy(vv, ps)
                            nc.sync.dma_start(
                                v_d[sc * P : (sc + 1) * P, n2 * TQ : (n2 + 1) * TQ], vv
                            )

            # ================= Phase 2: attention per head =================
            with tc.tile_pool(name="ph2", bufs=2) as ph2:
                for h in range(H):
                    kT_h = ph2.tile([DH, T], bf, tag="kT_h")
                    nc.sync.dma_start(kT_h, kT_d[h * DH : (h + 1) * DH, :])
                    q_h = ph2.tile([DH, TQ], bf, tag="q_h")
                    nc.sync.dma_start(q_h, qT_d[h * DH : (h + 1) * DH, :])
                    v_ones = ph2.tile([P, SC, DH + 1], bf, tag="v_ones")
                    nc.any.memset(v_ones[:, :, DH], 1.0)
                    nc.sync.dma_start(
                        v_ones[:, :, 0:DH],
                        v_d.rearrange("(o p) e -> p o e", p=P)[
                            :, :, h * DH : (h + 1) * DH
                        ],
                    )

                    p_sb = ph2.tile([P, SC, TQ], bf, tag="p_sb")
                    for sc in range(SC):
                        ps_l = psum.tile([P, TQ], f32, tag="ps_main")
                        nc.tensor.matmul(
                            ps_l,
                            kT_h[:, sc * P : (sc + 1) * P],
                            q_h,
                            start=True,
                            stop=True,
                        )
                        # p = exp(logits / sqrt(dh)) * mask
                        ptmp = ph2.tile([P, TQ], bf, tag="ptmp")
                        nc.scalar.activation(
                            ptmp, ps_l, AF.Exp, scale=float(DH**-0.5)
                        )
                        nc.vector.tensor_mul(
                            p_sb[:, sc, :], ptmp, maskT_sb[:, sc, :]
                        )

                    ps_av = psum_av.tile([DH + 1, TQ], f32, tag="ps_av")
                    for sc in range(SC):
                        nc.tensor.matmul(
                            ps_av,
                            v_ones[:, sc, :],
                            p_sb[:, sc, :],
                            start=(sc == 0),
                            stop=(sc == SC - 1),
                        )
                    # stash unnormalized head output + its denominator row
                    nc.any.tensor_copy(
                        hT_sb[(h % 2) * DH : (h % 2) * DH + DH, h // 2, :],
                        ps_av[0:DH, :],
                    )
                    # engines can't write at partition offset h; bounce via
                    # partition 0 then SBUF->SBUF DMA (partition-granular)
                    dtmp = ph2.tile([1, TQ], f32, tag="dtmp")
                    nc.any.tensor_copy(dtmp, ps_av[DH : DH + 1, :])
                    nc.sync.dma_start(denom_sb[h : h + 1, :], dtmp)

            # ================= Phase 3: normalize + residual + FFN =========
            with tc.tile_pool(name="ph3", bufs=3) as ph3, tc.tile_pool(
                name="ph3a", bufs=1
            ) as ph3a:
                recip_all = ph3a.tile([H, TQ], f32r)
                nc.vector.reciprocal(recip_all, denom_sb)

                xq_sb2 = ph3a.tile([P, DO, TQ], bf)
                nc.sync.dma_start(xq_sb2, xqT[:, :].rearrange("(o p) t -> p o t", p=P))
                for dc in range(DO):
                    # broadcast 1/denom to the 128 partitions of this d-chunk
                    ps_bc = psum_b.tile([P, TQ], f32, tag="ps_bc")
                    nc.tensor.matmul(
                        ps_bc,
                        sel_sb[:, dc * P : (dc + 1) * P],
                        recip_all,
                        start=True,
                        stop=True,
                    )
                    nc.vector.tensor_mul(hT_sb[:, dc, :], hT_sb[:, dc, :], ps_bc)
                    nc.vector.tensor_add(
                        hT_sb[:, dc, :], hT_sb[:, dc, :], xq_sb2[:, dc, :]
                    )

                # ff1: aT[m, tq] = gelu(w1T.T @ hT + b1)
                aT_sb = ph3a.tile([P, MC, TQ], bf)
                w1r = w1T[:, :].rearrange("(o p) m -> p o m", p=P)
                for mc in range(MC):
                    w1_sb = ph3.tile([P, DO, P], bf, tag="w1_sb")
                    nc.sync.dma_start(w1_sb, w1r[:, :, mc * P : (mc + 1) * P])
                    ps = psum.tile([P, TQ], f32, tag="ps_main")
                    for dc in range(DO):
                        nc.tensor.matmul(
                            ps,
                            w1_sb[:, dc, :],
                            hT_sb[:, dc, :],
                            start=(dc == 0),
                            stop=(dc == DO - 1),
                        )
                    nc.scalar.activation(
                        aT_sb[:, mc, :], ps, AF.Gelu, bias=b1_sb[:, mc : mc + 1]
                    )

                # ff2: out[tq, d] = aT.T @ w2T + x_res
                xres_r = x_res[:, :].rearrange("(o p) d -> p o d", p=P)
                w2r = w2T[:, :].rearrange("(o p) d -> p o d", p=P)
                for tqc in range(TQ // P):
                    xr_sb = ph3.tile([P, D], f32, tag="xr_sb")
                    nc.sync.dma_start(xr_sb, xres_r[:, tqc, :])
                    for n2 in range(D // TQ):
                        ps = psum.tile([P, TQ], f32, tag="ps_main")
                        for mc in range(MC):
                            w2_sb = ph3.tile([P, TQ], bf, tag="w2_sb")
                            nc.sync.dma_start(
                                w2_sb, w2r[:, mc, n2 * TQ : (n2 + 1) * TQ]
                            )
                            nc.tensor.matmul(
                                ps,
                                aT_sb[:, mc, tqc * P : (tqc + 1) * P],
                                w2_sb,
                                start=(mc == 0),
                                stop=(mc == MC - 1),
                            )
                        o_sb = ph3.tile([P, TQ], f32, tag="o_sb")
                        nc.vector.tensor_add(
                            o_sb, ps, xr_sb[:, n2 * TQ : (n2 + 1) * TQ]
                        )
                        nc.sync.dma_start(
                            out[tqc * P : (tqc + 1) * P, n2 * TQ : (n2 + 1) * TQ],
                            o_sb,
                        )

    nc.compile()
    return nc


_nc_cache = None


def _get_nc():
    global _nc_cache
    if _nc_cache is None:
        _nc_cache = _build()
    return _nc_cache


def _make_sel():
    sel = np.zeros((H, D), np.float32)
    for dc in range(DO):
        for p in range(P):
            sel[2 * dc + p // DH, dc * P + p] = 1.0
    return sel


def _prepare_in_maps(x, mask, wq, wk, wv, w1, b1, w2, b2):
    f = np.float32
    c = np.ascontiguousarray

    def cb(a):  # to contiguous bf16
        return c(np.asarray(a, np.float32).astype(BF16))

    wqT = cb(wq.transpose(2, 0, 1).reshape(D, D))
    wkT = cb(wk.transpose(2, 0, 1).reshape(D, D))
    wvT = cb(wv.transpose(2, 0, 1).reshape(D, D))
    w1T = cb(w1.T)
    w2T = cb(w2.T)
    b1c = c(np.asarray(b1, f).reshape(MC, P).T)
    shared = dict(
        wqT=wqT, wkT=wkT, wvT=wvT, w1T=w1T, w2T=w2T, b1c=b1c, sel_in=_make_sel()
    )

    in_maps = []
    for core in range(NCORES):
        b, jq = divmod(core, 4)
        q0 = jq * TQ
        xT_b = cb(np.asarray(x[b], f).T)
        m = dict(shared)
        m["xT"] = xT_b
        m["xqT"] = c(xT_b[:, q0 : q0 + TQ])
        m["x_res"] = c(np.asarray(x[b, q0 : q0 + TQ, :], f) + np.asarray(b2, f)[None, :])
        m["maskT"] = cb(np.asarray(mask[q0 : q0 + TQ, :]).T)
        in_maps.append(m)
    return in_maps


def _run(inputs, trace=False):
    from concourse import bass_utils

    nc = _get_nc()
    in_maps = _prepare_in_maps(**inputs)
    res = bass_utils.run_bass_kernel_spmd(
        nc, in_maps, core_ids=list(range(NCORES)), trace=trace
    )
    out = np.empty((B, T, D), np.float32)
    for core in range(NCORES):
        b, jq = divmod(core, 4)
        out[b, jq * TQ : (jq + 1) * TQ, :] = res.results[core]["out"]
    return out, res


def kernel(**inputs):
    inputs = {k: np.asarray(v) for k, v in inputs.items()}
    out, _ = _run(inputs, trace=False)
    return out


# revision 10
# speedup vs baseline: 1.9984x; 1.5977x over previous
"""Trainium2 Bass kernel for nn_AttentionBlock (B=2, T=2048, D=1024, H=16).

Sharding: 8 cores = 2 batches x 4 query-blocks of 512 tokens.
Each core: recomputes K/V projection for its batch (full 2048 kv tokens),
Q projection + attention + FFN for its 512-token query block. Zero
collectives; host gathers the 8 [512, 1024] output shards.

Matmul inputs are bf16 (1 cycle/row on PE, fp32 PSUM accumulation);
residual paths stay fp32. Orientation scheme keeps everything transposed
so no on-device transposes are needed:
  kT/qT: [head_dim, tokens]    logitsT: [kv, q]   p = exp(logitsT)*maskT
  v:     [kv, head_dim(+ones)] av: [head_dim+1, q] (row 64 = softmax denom)
  hT:    [d, q]                aT(ffn): [dff, q]   out: [q, d]
Softmax denominators for all 16 heads are normalized in one batch
(single RECIPROCAL + 8 selector-matmul broadcasts). w1/w2 are streamed
exactly once; ff2 accumulates 4 query-chunks in parallel PSUM banks.
"""

import sys

for _p in ("/opt/trn_rl_repo",):
    if _p not in sys.path:
        sys.path.insert(0, _p)

import ml_dtypes
import numpy as np

B, T, D, H = 2, 2048, 1024, 16
DH = D // H  # 64
DFF = 4 * D  # 4096
P = 128
TQ = 512  # query tokens per core
NCORES = 8
DO = D // P  # 8 d-chunks
SC = T // P  # 16 kv chunks
MC = DFF // P  # 32 ff chunks

BF16 = ml_dtypes.bfloat16


def _build():
    import concourse.bass as bass
    import concourse.mybir as mybir
    import concourse.tile as tile
    from concourse import bacc

    f32 = mybir.dt.float32
    f32r = mybir.dt.float32r
    bf = mybir.dt.bfloat16
    AF = mybir.ActivationFunctionType

    nc = bacc.Bacc("TRN2", target_bir_lowering=False)

    # ---- I/O ----
    xT = nc.dram_tensor("xT", [D, T], bf, kind="ExternalInput")
    xqT = nc.dram_tensor("xqT", [D, TQ], bf, kind="ExternalInput")
    x_res = nc.dram_tensor("x_res", [TQ, D], f32, kind="ExternalInput")
    maskT = nc.dram_tensor("maskT", [T, TQ], bf, kind="ExternalInput")
    wqT = nc.dram_tensor("wqT", [D, D], bf, kind="ExternalInput")
    wkT = nc.dram_tensor("wkT", [D, D], bf, kind="ExternalInput")
    wvT = nc.dram_tensor("wvT", [D, D], bf, kind="ExternalInput")
    w1T = nc.dram_tensor("w1T", [D, DFF], bf, kind="ExternalInput")
    w2T = nc.dram_tensor("w2T", [DFF, D], bf, kind="ExternalInput")
    b1c = nc.dram_tensor("b1c", [P, MC], f32, kind="ExternalInput")
    # sel[h, dc*128+p] = 1 if head-of-partition-p-in-chunk-dc == h (f32r path)
    sel_in = nc.dram_tensor("sel_in", [H, D], f32r, kind="ExternalInput")
    out = nc.dram_tensor("out", [TQ, D], f32, kind="ExternalOutput")

    with tile.TileContext(nc) as tc, nc.allow_low_precision(
        reason="bf16 matmul inputs; all accumulation in fp32 PSUM"
    ):
        with (
            tc.tile_pool(name="dram", bufs=1, space="DRAM") as dram,
            tc.tile_pool(name="resident", bufs=1) as resident,
        ):
            kT_d = dram.tile([D, T], bf)
            qT_d = dram.tile([D, TQ], bf)
            v_d = dram.tile([T, D], bf)

            sel_sb = resident.tile([H, D], f32r)
            nc.sync.dma_start(sel_sb, sel_in[:, :])
            b1_sb = resident.tile([P, MC], f32)
            nc.sync.dma_start(b1_sb, b1c[:, :])

            maskT_sb = resident.tile([P, SC, TQ], bf)
            nc.sync.dma_start(maskT_sb, maskT[:, :].rearrange("(o p) t -> p o t", p=P))
            xq_sb = resident.tile([P, DO, TQ], bf)
            nc.sync.dma_start(xq_sb, xqT[:, :].rearrange("(o p) t -> p o t", p=P))
            # hT: unnormalized per-head attn outputs, then normalized + residual
            hT_sb = resident.tile([P, DO, TQ], bf)
            denom_sb = resident.tile([H, TQ], f32)
            # double-buffered per-head v tiles; ones column written once
            vo_tiles = [
                resident.tile([P, SC, DH + 1], bf, name=f"vo{i}") for i in range(2)
            ]
            for vo in vo_tiles:
                nc.vector.memset(vo[:, :, DH], 1.0)

            # ================= Phase 1: K/Q/V projections =================
            with (
                tc.tile_pool(name="ph1", bufs=3) as ph1,
                tc.tile_pool(name="ph1w", bufs=1) as ph1w,
                tc.tile_pool(name="psum1", bufs=6, space="PSUM") as psum1,
            ):
                xT_sb = ph1w.tile([P, DO, T], bf)
                nc.sync.dma_start(xT_sb, xT[:, :].rearrange("(o p) t -> p o t", p=P))
                wk_sb = ph1w.tile([P, DO, D], bf)
                nc.sync.dma_start(wk_sb, wkT[:, :].rearrange("(o p) e -> p o e", p=P))
                wq_sb = ph1w.tile([P, DO, D], bf)
                nc.sync.dma_start(wq_sb, wqT[:, :].rearrange("(o p) e -> p o e", p=P))
                wv_sb = ph1w.tile([P, DO, D], bf)
                nc.sync.dma_start(wv_sb, wvT[:, :].rearrange("(o p) e -> p o e", p=P))

                # K: kT[he, s], he-chunk-major so early heads finish first
                for hc in range(DO):
                    for s4 in range(T // TQ):
                        ps = psum1.tile([P, TQ], f32, tag="ps_main")
                        for dc in range(DO):
                            nc.tensor.matmul(
                                ps,
                                wk_sb[:, dc, hc * P : (hc + 1) * P],
                                xT_sb[:, dc, s4 * TQ : (s4 + 1) * TQ],
                                start=(dc == 0),
                                stop=(dc == DO - 1),
                            )
                        kv = ph1.tile([P, TQ], bf, tag="kstage")
                        nc.vector.tensor_copy(kv, ps)
                        nc.sync.dma_start(
                            kT_d[hc * P : (hc + 1) * P, s4 * TQ : (s4 + 1) * TQ], kv
                        )
                    # Q for the same head-chunk right after its K
                    ps = psum1.tile([P, TQ], f32, tag="ps_main")
                    for dc in range(DO):
                        nc.tensor.matmul(
                            ps,
                            wq_sb[:, dc, hc * P : (hc + 1) * P],
                            xq_sb[:, dc, :],
                            start=(dc == 0),
                            stop=(dc == DO - 1),
                        )
                    qv = ph1.tile([P, TQ], bf, tag="qstage")
                    nc.vector.tensor_copy(qv, ps)
                    nc.sync.dma_start(qT_d[hc * P : (hc + 1) * P, :], qv)

                # V: v[s, he]
                for sc in range(SC):
                    for n2 in range(D // TQ):
                        ps = psum1.tile([P, TQ], f32, tag="ps_main")
                        for dc in range(DO):
                            nc.tensor.matmul(
                                ps,
                                xT_sb[:, dc, sc * P : (sc + 1) * P],
                                wv_sb[:, dc, n2 * TQ : (n2 + 1) * TQ],
                                start=(dc == 0),
                                stop=(dc == DO - 1),
                            )
                        vv = ph1.tile([P, TQ], bf, tag="vstage")
                        nc.vector.tensor_copy(vv, ps)
                        nc.sync.dma_start(
                            v_d[sc * P : (sc + 1) * P, n2 * TQ : (n2 + 1) * TQ], vv
                        )

            # ================= Phase 2: attention per head =================
            with (
                tc.tile_pool(name="ph2", bufs=2) as ph2,
                tc.tile_pool(name="ph2t", bufs=4) as ph2t,
                tc.tile_pool(name="psum2", bufs=3, space="PSUM") as psum2,
                tc.tile_pool(name="psum2av", bufs=2, space="PSUM") as psum2av,
            ):
                for h in range(H):
                    kT_h = ph2.tile([DH, T], bf, tag="kT_h")
                    nc.sync.dma_start(kT_h, kT_d[h * DH : (h + 1) * DH, :])
                    q_h = ph2.tile([DH, TQ], bf, tag="q_h")
                    nc.sync.dma_start(q_h, qT_d[h * DH : (h + 1) * DH, :])
                    vo = vo_tiles[h % 2]
                    nc.sync.dma_start(
                        vo[:, :, 0:DH],
                        v_d.rearrange("(o p) e -> p o e", p=P)[
                            :, :, h * DH : (h + 1) * DH
                        ],
                    )

                    p_sb = ph2.tile([P, SC, TQ], bf, tag="p_sb")
                    for sc2 in range(SC // 2):
                        ps_l = psum2.tile([P, 2, TQ], f32, tag="ps_l")
                        for j in range(2):
                            nc.tensor.matmul(
                                ps_l[:, j, :],
                                kT_h[:, (2 * sc2 + j) * P : (2 * sc2 + j + 1) * P],
                                q_h,
                                start=True,
                                stop=True,
                            )
                        # p = exp(logits / sqrt(dh)) * mask  (2 chunks fused)
                        ptmp = ph2t.tile([P, 2, TQ], bf, tag="ptmp")
                        nc.scalar.activation(
                            ptmp, ps_l, AF.Exp, scale=float(DH**-0.5)
                        )
                        nc.vector.tensor_mul(
                            p_sb[:, 2 * sc2 : 2 * sc2 + 2, :],
                            ptmp,
                            maskT_sb[:, 2 * sc2 : 2 * sc2 + 2, :],
                        )

                    ps_av = psum2av.tile([DH + 1, TQ], f32, tag="ps_av")
                    for sc in range(SC):
                        nc.tensor.matmul(
                            ps_av,
                            vo[:, sc, :],
                            p_sb[:, sc, :],
                            start=(sc == 0),
                            stop=(sc == SC - 1),
                        )
                    # stash unnormalized head output + its denominator row
                    nc.vector.tensor_copy(
                        hT_sb[(h % 2) * DH : (h % 2) * DH + DH, h // 2, :],
                        ps_av[0:DH, :],
                    )
                    # engines can't write at partition offset h; bounce via
                    # partition 0 then SBUF->SBUF DMA (partition-granular)
                    dtmp = ph2t.tile([1, TQ], f32, tag="dtmp")
                    nc.vector.tensor_copy(dtmp, ps_av[DH : DH + 1, :])
                    nc.sync.dma_start(denom_sb[h : h + 1, :], dtmp)

            # ================= Phase 3: normalize + residual + FFN =========
            with (
                tc.tile_pool(name="ph3", bufs=3) as ph3,
                tc.tile_pool(name="ph3a", bufs=1) as ph3a,
                tc.tile_pool(name="psum3f", bufs=2, space="PSUM") as psum3f,
                tc.tile_pool(name="psum3b", bufs=2, space="PSUM") as psum3b,
                tc.tile_pool(name="psum3ff2", bufs=1, space="PSUM") as psum3ff2,
            ):
                recip_all = ph3a.tile([H, TQ], f32r)
                nc.vector.reciprocal(recip_all, denom_sb)

                for dc in range(DO):
                    # broadcast 1/denom to the 128 partitions of this d-chunk
                    ps_bc = psum3b.tile([P, TQ], f32, tag="ps_bc")
                    nc.tensor.matmul(
                        ps_bc,
                        sel_sb[:, dc * P : (dc + 1) * P],
                        recip_all,
                        start=True,
                        stop=True,
                    )
                    nc.vector.tensor_mul(hT_sb[:, dc, :], hT_sb[:, dc, :], ps_bc)
                    nc.vector.tensor_add(
                        hT_sb[:, dc, :], hT_sb[:, dc, :], xq_sb[:, dc, :]
                    )

                # ff1: aT[m, tq] = gelu(w1T.T @ hT + b1); separate tiles so
                # ff2 chains can start as soon as each aT chunk lands
                aT_tiles = []
                w1r = w1T[:, :].rearrange("(o p) m -> p o m", p=P)
                for mc in range(MC):
                    w1_sb = ph3.tile([P, DO, P], bf, tag="w1_sb")
                    nc.sync.dma_start(w1_sb, w1r[:, :, mc * P : (mc + 1) * P])
                    ps = psum3f.tile([P, TQ], f32, tag="ps_f1")
                    for dc in range(DO):
                        nc.tensor.matmul(
                            ps,
                            w1_sb[:, dc, :],
                            hT_sb[:, dc, :],
                            start=(dc == 0),
                            stop=(dc == DO - 1),
                        )
                    aT = ph3a.tile([P, TQ], bf, tag=f"aT{mc}")
                    nc.scalar.activation(
                        aT, ps, AF.Gelu, bias=b1_sb[:, mc : mc + 1]
                    )
                    aT_tiles.append(aT)

                # ff2: out[tq, d] = aT.T @ w2T + x_res; w2 streamed once,
                # 4 query-chunks accumulate in parallel PSUM banks
                xr_sb = ph3a.tile([P, TQ // P, D], f32)
                nc.sync.dma_start(
                    xr_sb, x_res[:, :].rearrange("(o p) d -> p o d", p=P)
                )
                w2r = w2T[:, :].rearrange("(o p) d -> p o d", p=P)
                for n2 in range(D // TQ):
                    psf = psum3ff2.tile([P, TQ // P, TQ], f32, tag="ps_ff2")
                    for mc in range(MC):
                        w2_sb = ph3.tile([P, TQ], bf, tag="w2_sb")
                        nc.sync.dma_start(
                            w2_sb, w2r[:, mc, n2 * TQ : (n2 + 1) * TQ]
                        )
                        for tqc in range(TQ // P):
                            nc.tensor.matmul(
                                psf[:, tqc, :],
                                aT_tiles[mc][:, tqc * P : (tqc + 1) * P],
                                w2_sb,
                                start=(mc == 0),
                                stop=(mc == MC - 1),
                            )
                    for tqc in range(TQ // P):
                        o_sb = ph3.tile([P, TQ], f32, tag="o_sb")
                        nc.vector.tensor_add(
                            o_sb,
                            psf[:, tqc, :],
                            xr_sb[:, tqc, n2 * TQ : (n2 + 1) * TQ],
                        )
                        nc.sync.dma_start(
                            out[tqc * P : (tqc + 1) * P, n2 * TQ : (n2 + 1) * TQ],
                            o_sb,
                        )

    nc.compile()
    return nc


_nc_cache = None


def _get_nc():
    global _nc_cache
    if _nc_cache is None:
        _nc_cache = _build()
    return _nc_cache


def _make_sel():
    sel = np.zeros((H, D), np.float32)
    for dc in range(DO):
        for p in range(P):
            sel[2 * dc + p // DH, dc * P + p] = 1.0
    return sel


def _prepare_in_maps(x, mask, wq, wk, wv, w1, b1, w2, b2):
    f = np.float32
    c = np.ascontiguousarray

    def cb(a):  # to contiguous bf16
        return c(np.asarray(a, np.float32).astype(BF16))

    wqT = cb(wq.transpose(2, 0, 1).reshape(D, D))
    wkT = cb(wk.transpose(2, 0, 1).reshape(D, D))
    wvT = cb(wv.transpose(2, 0, 1).reshape(D, D))
    w1T = cb(w1.T)
    w2T = cb(w2.T)
    b1c = c(np.asarray(b1, f).reshape(MC, P).T)
    shared = dict(
        wqT=wqT, wkT=wkT, wvT=wvT, w1T=w1T, w2T=w2T, b1c=b1c, sel_in=_make_sel()
    )

    in_maps = []
    for core in range(NCORES):
        b, jq = divmod(core, 4)
        q0 = jq * TQ
        xT_b = cb(np.asarray(x[b], f).T)
        m = dict(shared)
        m["xT"] = xT_b
        m["xqT"] = c(xT_b[:, q0 : q0 + TQ])
        m["x_res"] = c(np.asarray(x[b, q0 : q0 + TQ, :], f) + np.asarray(b2, f)[None, :])
        m["maskT"] = cb(np.asarray(mask[q0 : q0 + TQ, :]).T)
        in_maps.append(m)
    return in_maps


def _run(inputs, trace=False):
    from concourse import bass_utils

    nc = _get_nc()
    in_maps = _prepare_in_maps(**inputs)
    res = bass_utils.run_bass_kernel_spmd(
        nc, in_maps, core_ids=list(range(NCORES)), trace=trace
    )
    out = np.empty((B, T, D), np.float32)
    for core in range(NCORES):
        b, jq = divmod(core, 4)
        out[b, jq * TQ : (jq + 1) * TQ, :] = res.results[core]["out"]
    return out, res


def kernel(**inputs):
    inputs = {k: np.asarray(v) for k, v in inputs.items()}
    out, _ = _run(inputs, trace=False)
    return out
